# revision 38
# baseline (speedup 1.0000x reference)
"""MaxK-GCN conv on 8 Trainium2 NeuronCores.

Pipeline (per core c, SPMD over 8 cores; nodes sharded 8 x 12500):
  phase 1: h = featT_c.T @ W (PE), top-16-of-64 threshold mask (DVE max8 +
           match_replace), scale by (max(out_deg,1)*max(in_deg,1))^-0.5,
           emit bf16 rows -> local table shard [12800, 64] bf16.
  One AllGather of the whole 64-col table (12.85MB out) into bigQ: the
  collective cost ramps strongly with message size, so a single large
  transfer beats overlapped quarter-chunks.  A per-quarter strided DMA
  then expands rows into 128-col tableG (upper half junk) because
  dma_gather elements must be 256B-aligned; the matmul only reads the
  first 64 columns, so the junk is never consumed.
  phase 2: slab-major edge aggregation (4 slabs keep gather tables under
  the 32768-row int16 index limit).  Edges with dst in shard c are
  host-sorted by (slab-of-src, dst-block): per slab, dma_gather src rows
  from tableG, build one-hot S tiles from dst values (DVE is_eq vs iota
  with a packed 2-elem last dim for the 2x perf mode), matmul S^T @ G
  into a rotating PSUM bank per (slab, block) cell, and fold each cell
  into an SBUF fp32 accumulator [128, 98, 64].  After slab 3's fold,
  DMA each block's rows to DRAM.

Edge bookkeeping (sort, padding, degree counts) is host-side index metadata;
all floating-point math runs on device.
"""
import sys
import os

sys.path.insert(0, "/opt/trn_rl_repo")

import numpy as np
import ml_dtypes
import concourse.bacc as bacc
import concourse.mybir as mybir
import concourse.tile as tile
from concourse.bass_utils import run_bass_kernel_spmd

P = 128
N_NODES = 100000
IN_FEATS = 256
OUT_FEATS = 64
N_CORES = 8
SHARD = N_NODES // N_CORES          # 12500 real nodes per core
SHARD_PAD = 12800                   # 100 * 128 table rows, quarter-aligned
N_BLOCKS = SHARD_PAD // P           # 100 table blocks
OUT_PAD = 12544                     # 98 * 128 dst rows
DBLOCKS = OUT_PAD // P              # 98 dst blocks
N_SLABS = 4
QBLOCKS = N_BLOCKS // N_SLABS       # table blocks per shard-quarter
assert QBLOCKS * N_SLABS == N_BLOCKS
QROWS = SHARD_PAD // N_SLABS        # 3200 rows per shard-quarter
SLAB_ROWS = N_CORES * QROWS         # 25600 rows per quarter-table
TABLE_COLS = OUT_FEATS              # 64 bf16 per table row
TILES_PER_CALL = 24                 # <= 24*128 idx per dma_gather call
SW = 16                             # one-hot tiles per DVE op
NEG_INF = -3.0e38


def _inspect(src, dst):
    """Host inspector: per-core sorted edge data + shared static tile grid."""
    src = src.astype(np.int64)
    dst = dst.astype(np.int64)
    core = dst // SHARD
    gidx_of, dstrel_of = [], []
    counts = np.zeros((N_CORES, N_SLABS, DBLOCKS), dtype=np.int64)
    for c in range(N_CORES):
        m = core == c
        s_c = src[m]
        d_loc = dst[m] - c * SHARD
        blk = d_loc >> 7
        s8 = s_c // SHARD
        local = s_c % SHARD
        slab = local // QROWS                 # quarter of the owning shard
        gidx = s8 * QROWS + (local - slab * QROWS)  # row within quarter-table
        order = np.lexsort((gidx, blk, slab))
        gidx_of.append(gidx[order])
        dstrel_of.append((d_loc - (blk << 7))[order])
        key = slab * DBLOCKS + blk
        cnt = np.bincount(key, minlength=N_SLABS * DBLOCKS)
        counts[c] = cnt.reshape(N_SLABS, DBLOCKS)
    T = ((counts + P - 1) // P).max(axis=0)  # shared tile grid [4, 98]
    assert T.min() >= 1
    return gidx_of, dstrel_of, counts, T


def _make_schedule(T):
    """Static tile stream + gather call list, identical on all cores.

    tile_meta: [slab, block, is_first_of_cell, is_last_of_cell]
    calls: (slab, first_tile, n_tiles), runs never cross a slab boundary.
    """
    tile_meta = []
    for s in range(N_SLABS):
        for b in range(DBLOCKS):
            n = T[s, b]
            for i in range(n):
                tile_meta.append([s, b, i == 0, i == n - 1])
    calls = []
    j, n = 0, len(tile_meta)
    while j < n:
        s = tile_meta[j][0]
        k = j
        while k < n and tile_meta[k][0] == s and k - j < TILES_PER_CALL:
            k += 1
        calls.append((s, j, k - j))
        j = k
    return tile_meta, calls


def _per_core_streams(c, tile_meta, counts, gidx_of, dstrel_of):
    """This core's padded gather-idx + dst_rel streams matching the grid."""
    ntiles = len(tile_meta)
    idx_stream = np.zeros(ntiles * P, dtype=np.int16)
    dst_stream = np.full(ntiles * P, -1.0, dtype=np.float32)
    edge_ptr = 0
    j = 0
    while j < ntiles:
        s, b = tile_meta[j][:2]
        k = j
        while k < ntiles and tile_meta[k][:2] == [s, b]:
            k += 1
        nseg = int(counts[c, s, b])
        base = j * P
        idx_stream[base:base + nseg] = gidx_of[c][edge_ptr:edge_ptr + nseg]
        dst_stream[base:base + nseg] = dstrel_of[c][edge_ptr:edge_ptr + nseg]
        edge_ptr += nseg
        j = k
    assert edge_ptr == len(gidx_of[c])
    idx_wrapped = np.tile(idx_stream.reshape(-1, 16).T, (8, 1)).copy()
    # [P, ntiles, 2]: per-tile dst value duplicated so the one-hot compare
    # can use a packed 2-elem last dim (DVE 2x mode) instead of stride-0
    dstv = dst_stream.reshape(ntiles, P).T          # [P, ntiles]
    dstv2 = np.repeat(dstv[:, :, None], 2, axis=2).reshape(P, ntiles * 2)
    return idx_wrapped, dstv2.copy()


def _build(tile_meta, calls):
    ntiles = len(tile_meta)
    nc = bacc.Bacc("TRN2", target_bir_lowering=False, num_swdge_queues=4)
    dt = mybir.dt

    featT = nc.declare_dram_parameter("featT", [IN_FEATS, SHARD_PAD], dt.float32, isOutput=False)
    w_in = nc.declare_dram_parameter("w", [IN_FEATS, OUT_FEATS], dt.float32, isOutput=False)
    biasb = nc.declare_dram_parameter("biasb", [P, OUT_FEATS], dt.float32, isOutput=False)
    idegw = nc.declare_dram_parameter("idegw", [P, N_BLOCKS], dt.float32, isOutput=False)
    odegw = nc.declare_dram_parameter("odegw", [P, N_BLOCKS], dt.float32, isOutput=False)
    iota_in = nc.declare_dram_parameter("iota", [P, P], dt.bfloat16, isOutput=False)
    idxs_in = nc.declare_dram_parameter("idxs", [P, ntiles * 8], dt.int16, isOutput=False)
    dstv_in = nc.declare_dram_parameter("dstv", [P, ntiles * 2], dt.bfloat16, isOutput=False)
    out_d = nc.declare_dram_parameter("out", [OUT_PAD, OUT_FEATS], dt.float32, isOutput=True)

    tableL = nc.dram_tensor("tableL", [SHARD_PAD, TABLE_COLS], dt.bfloat16)
    # gather elements must be 256B, but collective outputs must be
    # contiguous: AllGather into 64-col tableQ, then a local strided DMA
    # expands each quarter into 128-col tableG rows (upper half junk that
    # the matmul never reads).
    # collective chunking (in units of 25-block quarters), big-first by
    # default: the collective cost model rewards larger transfers, while
    # gather slabs must stay <=32768 rows for int16 indices; the expansion
    # pass decouples the two.
    CC_PLAN = [4]
    assert sum(CC_PLAN) == N_SLABS
    chunk_of_q, off_of_q = {}, {}
    q0 = 0
    for k, n in enumerate(CC_PLAN):
        for o in range(n):
            chunk_of_q[q0 + o] = k
            off_of_q[q0 + o] = o
        q0 += n
    bigQ = [nc.dram_tensor(f"bigQ{k}", [N_CORES * n * QROWS, TABLE_COLS],
                           dt.bfloat16, addr_space="Shared")
            for k, n in enumerate(CC_PLAN)]
    tableG = [nc.dram_tensor(f"tableG{q}", [SLAB_ROWS, 2 * TABLE_COLS],
                             dt.bfloat16) for q in range(N_SLABS)]

    with tile.TileContext(nc) as tc:
        with tc.tile_pool(name="const", bufs=1) as constp, \
             tc.tile_pool(name="gp", bufs=10) as gp, \
             tc.tile_pool(name="sp", bufs=8) as sps, \
             tc.tile_pool(name="accs", bufs=1) as accs:

            # ---- constants ----
            w_sb = constp.tile([P, 2, OUT_FEATS], dt.float32)
            for k in range(2):
                nc.sync.dma_start(out=w_sb[:, k, :], in_=w_in[k * P:(k + 1) * P, :])
            bias_sb = constp.tile([P, OUT_FEATS], dt.float32)
            nc.sync.dma_start(out=bias_sb[:], in_=biasb[:])
            iota_sb = constp.tile([P, 1, P], dt.bfloat16)
            nc.sync.dma_start(out=iota_sb[:, 0, :], in_=iota_in[:])
            dstv_sb = constp.tile([P, ntiles, 1, 2], dt.bfloat16)
            nc.sync.dma_start(out=dstv_sb[:, :, 0, :], in_=dstv_in[:])
            idx_sb = constp.tile([P, ntiles * 8], dt.int16)
            nc.sync.dma_start(out=idx_sb[:], in_=idxs_in[:])

            acc_sb = accs.tile([P, DBLOCKS, OUT_FEATS], dt.float32)


            # ---- phase 1: table build (pools scoped to free SBUF/PSUM) ----
            with tc.tile_pool(name="ft", bufs=1) as ftp, \
                 tc.tile_pool(name="ph1", bufs=4) as ph1, \
                 tc.tile_pool(name="ph1ps", bufs=4, space="PSUM") as ph1ps:

                ideg_sb = ph1.tile([P, N_BLOCKS], dt.float32, tag="deg")
                odeg_sb = ph1.tile([P, N_BLOCKS], dt.float32, tag="deg")
                nc.sync.dma_start(out=ideg_sb[:], in_=idegw[:])
                nc.sync.dma_start(out=odeg_sb[:], in_=odegw[:])
                scale_sb = constp.tile([P, N_BLOCKS], dt.float32)
                nc.vector.tensor_scalar_max(ideg_sb[:], ideg_sb[:], 1.0)
                nc.vector.tensor_scalar_max(odeg_sb[:], odeg_sb[:], 1.0)
                nc.vector.tensor_mul(out=scale_sb[:], in0=ideg_sb[:], in1=odeg_sb[:])
                nc.scalar.activation(out=scale_sb[:], in_=scale_sb[:],
                                     func=mybir.ActivationFunctionType.Sqrt)
                nc.vector.reciprocal(out=scale_sb[:], in_=scale_sb[:])

                # featT in chunks (2 k-chunks x 8 column chunks)
                FCH = [13] * 7 + [9]
                FBASE = [0, 13, 26, 39, 52, 65, 78, 91]
                ft_sb = {}
                for fc in range(8):
                    for k in range(2):
                        t_ = ftp.tile([P, FCH[fc] * P], dt.float32, tag=f"ft{k}", bufs=3)
                        feng = nc.sync if (fc * 2 + k) % 2 == 0 else nc.scalar
                        feng.dma_start(
                            out=t_[:],
                            in_=featT[k * P:(k + 1) * P,
                                      FBASE[fc] * P:(FBASE[fc] + FCH[fc]) * P])
                        ft_sb[(fc, k)] = t_

                for t in range(N_BLOCKS):
                    fc = min(t // 13, 7)
                    tc_rel = t - FBASE[fc]
                    hp = ph1ps.tile([P, OUT_FEATS], dt.float32, tag="hps")
                    for k in range(2):
                        nc.tensor.matmul(
                            out=hp[:],
                            lhsT=ft_sb[(fc, k)][:, tc_rel * P:(tc_rel + 1) * P],
                            rhs=w_sb[:, k, :],
                            start=(k == 0), stop=(k == 1),
                        )
                    h = ph1.tile([P, OUT_FEATS], dt.float32, tag="h")
                    nc.vector.tensor_copy(out=h[:], in_=hp[:])
                    m1 = ph1.tile([P, 8], dt.float32, tag="m1")
                    nc.vector.max(m1[:], h[:])
                    hneg = ph1.tile([P, OUT_FEATS], dt.float32, tag="hneg")
                    nc.vector.match_replace(out=hneg[:], in_to_replace=m1[:],
                                            in_values=h[:], imm_value=NEG_INF)
                    m2 = ph1.tile([P, 8], dt.float32, tag="m2")
                    nc.vector.max(m2[:], hneg[:])
                    # hm = (h >= thr) * h  in one fused op
                    hm = ph1.tile([P, OUT_FEATS], dt.float32, tag="mask")
                    nc.vector.scalar_tensor_tensor(
                        out=hm[:], in0=h[:], scalar=m2[:, 7:8], in1=h[:],
                        op0=mybir.AluOpType.is_ge, op1=mybir.AluOpType.mult)
                    # table row = bf16(hm * scale) via ACT's fused input scale
                    ttile = ph1.tile([P, TABLE_COLS], dt.bfloat16, tag="ttile")
                    nc.scalar.activation(out=ttile[:], in_=hm[:],
                                         func=mybir.ActivationFunctionType.Copy,
                                         scale=scale_sb[:, t:t + 1])
                    eng = nc.sync if t % 2 == 0 else nc.scalar
                    eng.dma_start(out=tableL[t * P:(t + 1) * P, :], in_=ttile[:])

                    # allgather chunk k as soon as its quarters are written
                    if (t + 1) % QBLOCKS == 0:
                        q = (t + 1) // QBLOCKS - 1
                        k = chunk_of_q[q]
                        if off_of_q[q] == CC_PLAN[k] - 1:
                            qk0 = q - CC_PLAN[k] + 1
                            nc.gpsimd.collective_compute(
                                "AllGather",
                                mybir.AluOpType.bypass,
                                replica_groups=[list(range(N_CORES))],
                                ins=[tableL[qk0 * QROWS:(q + 1) * QROWS, :]],
                                outs=[bigQ[k][:]],
                            )


            # ---- phase 2: slab-major edge aggregation ----
            # The tile scheduler's greedy list-scheduling entangles phase-2
            # work with phase 1 through the per-proc counting semaphores
            # (work hoisted into phase-1 idle slots transitively waits on
            # collectives and stalls everything behind it).  tile_wait_until
            # pins each slab's work to the expected completion time of its
            # quarter's AllGather, so the emitted order matches the real
            # timeline.
            PH1_MS = 0.085

            def cc_dur_us(nq):
                by = N_CORES * nq * QROWS * TABLE_COLS * 2
                lo, hi = float(1 << 23), 0.9 * float(1 << 25)
                tt = min(max((by - lo), 0.0) / (hi - lo), 1.0)
                bw = (1 - tt) * 40e9 + tt * 128e9 * 0.86
                return 15.0 + by / bw * 1e6

            # chunk completion estimates (ph1 block pace ~0.55us + issue dep)
            end_of_chunk = []
            prev = 0.0
            qq = 0
            for k, n in enumerate(CC_PLAN):
                qq += n
                issue = 0.030 + 0.014 * qq * QBLOCKS / 25
                prev = max(prev, issue) + cc_dur_us(n) / 1000
                end_of_chunk.append(prev)
            CC_MS = [end_of_chunk[chunk_of_q[q]] for q in range(N_SLABS)]

            phase2_stack = __import__("contextlib").ExitStack()
            accp = phase2_stack.enter_context(
                tc.tile_pool(name="accp", bufs=4, space="PSUM"))

            slab_range = {}
            for j, (s, b, st, sp_) in enumerate(tile_meta):
                if s not in slab_range:
                    slab_range[s] = [j, j]
                slab_range[s][1] = j + 1
            calls_of_slab = {s: [] for s in range(N_SLABS)}
            for ci, (s, j0, ct) in enumerate(calls):
                calls_of_slab[s].append((ci, j0, ct))

            g_tiles = {}
            s_tiles = {}
            for s in range(N_SLABS):
                j0s, j1s = slab_range[s]
                # one-hot builds for slab s: no data deps, prefetched during
                # the previous quarter's collective window
                for _one in [0]:
                    for j0 in range(j0s, j1s, SW):
                        jn = min(SW, j1s - j0)
                        s4 = sps.tile([P, SW, 64, 2], dt.bfloat16, tag="s")
                        nc.vector.tensor_tensor(
                            out=s4[:, :jn, :, :],
                            in0=dstv_sb[:, j0:j0 + jn, :, :]
                                .to_broadcast([P, jn, 64, 2]),
                            in1=iota_sb[:]
                                .rearrange("p o (a b) -> p o a b", b=2)
                                .to_broadcast([P, jn, 64, 2]),
                            op=mybir.AluOpType.is_equal)
                        for t in range(jn):
                            s_tiles[j0 + t] = (s4, t)

                for _one in [0]:
                    # expand quarter s for 256B gathers; from the ACT queue
                    # (idle after phase 1).  bigQ chunk layout: row of
                    # (core c, quarter-offset o, local r) is
                    # c*(n*QROWS) + o*QROWS + r; one DMA per core keeps
                    # each transfer under the 16384-descriptor cap.
                    k = chunk_of_q[s]
                    n = CC_PLAN[k]
                    o = off_of_q[s]
                    for c in range(N_CORES):
                        base = c * n * QROWS + o * QROWS
                        nc.scalar.dma_start(
                            out=tableG[s][c * QROWS:(c + 1) * QROWS,
                                          0:TABLE_COLS],
                            in_=bigQ[k][base:base + QROWS, :])
                    for ci, j0, ct in calls_of_slab[s]:
                        g = gp.tile([P, TILES_PER_CALL, 2 * TABLE_COLS],
                                    dt.bfloat16, tag="g")
                        nc.gpsimd.dma_gather(
                            out_ap=g[:, :ct, :],
                            in_ap=tableG[s][:],
                            idxs_ap=idx_sb[:, j0 * 8:(j0 + ct) * 8],
                            num_idxs=ct * P,
                            num_idxs_reg=ct * P,
                            elem_size=2 * TABLE_COLS,
                            single_packet=False,
                            queue_num=ci % 4,
                        )
                        for t in range(ct):
                            g_tiles[j0 + t] = (g, t)

                    acc = None
                    for j in range(j0s, j1s):
                        s_, b, st, sp_ = tile_meta[j]
                        if st:
                            acc = accp.tile([P, 512], dt.float32, tag="cell")
                        s4_t, s4_i = s_tiles[j]
                        g, gt = g_tiles[j]
                        nc.tensor.matmul(
                            out=acc[:, 0:OUT_FEATS],
                            lhsT=s4_t[:, s4_i, :, :].rearrange("p a b -> p (a b)"),
                            rhs=g[:, gt, 0:OUT_FEATS],
                            start=bool(st), stop=bool(sp_),
                            skip_group_check=True,
                        )
                        if sp_:
                            if s == 0:
                                nc.vector.tensor_add(out=acc_sb[:, b, :],
                                                     in0=acc[:, 0:OUT_FEATS],
                                                     in1=bias_sb[:])
                            else:
                                nc.vector.tensor_add(out=acc_sb[:, b, :],
                                                     in0=acc_sb[:, b, :],
                                                     in1=acc[:, 0:OUT_FEATS])
                            if s == N_SLABS - 1:
                                nc.sync.dma_start(
                                    out=out_d[b * P:(b + 1) * P, :],
                                    in_=acc_sb[:, b, :])
            phase2_stack.close()

    nc.finalize()
    return nc


def prepare(feat, weight, bias, src, dst):
    """Host prep + bass build: returns (nc, in_maps) ready to run."""
    feat = np.asarray(feat, dtype=np.float32)
    weight = np.asarray(weight, dtype=np.float32)
    bias = np.asarray(bias, dtype=np.float32)
    src = np.asarray(src)
    dst = np.asarray(dst)

    gidx_of, dstrel_of, counts, T = _inspect(src, dst)
    tile_meta, calls = _make_schedule(T)

    in_deg = np.bincount(dst, minlength=N_NODES).astype(np.float32)
    out_deg = np.bincount(src, minlength=N_NODES).astype(np.float32)
    ft = feat.T  # [256, 100000]
    iota = np.tile(np.arange(P, dtype=np.float32), (P, 1)).astype(ml_dtypes.bfloat16)

    in_maps = []
    for c in range(N_CORES):
        lo, hi = c * SHARD, (c + 1) * SHARD
        featT_c = np.zeros((IN_FEATS, SHARD_PAD), dtype=np.float32)
        featT_c[:, :SHARD] = ft[:, lo:hi]
        ideg_c = np.ones(SHARD_PAD, dtype=np.float32)
        odeg_c = np.ones(SHARD_PAD, dtype=np.float32)
        ideg_c[:SHARD] = in_deg[lo:hi]
        odeg_c[:SHARD] = out_deg[lo:hi]
        idx_wrapped, dstv2 = _per_core_streams(c, tile_meta, counts,
                                               gidx_of, dstrel_of)
        in_maps.append({
            "featT": featT_c,
            "w": weight,
            "biasb": np.tile(bias[None, :], (P, 1)).astype(np.float32),
            "idegw": ideg_c.reshape(N_BLOCKS, P).T.copy(),
            "odegw": odeg_c.reshape(N_BLOCKS, P).T.copy(),
            "iota": iota,
            "idxs": idx_wrapped,
            "dstv": dstv2.astype(ml_dtypes.bfloat16),
        })

    nc = _build(tile_meta, calls)
    return nc, in_maps


def kernel(feat, weight, bias, src, dst):
    nc, in_maps = prepare(feat, weight, bias, src, dst)
    res = run_bass_kernel_spmd(nc, in_maps, list(range(N_CORES)))
    out = np.concatenate(
        [res.results[c]["out"][:SHARD] for c in range(N_CORES)], axis=0)
    return out.astype(np.float32)
